# revision 8
# baseline (speedup 1.0000x reference)
"""CrossAttention (B=2, N=M=2048, 16 heads x 64) on 8 TRN2 NeuronCores.

Sharding: data-parallel over batch (2) x tensor-parallel over heads (4 per
core). Each core computes q/k/v projections for its 4 heads, streaming
softmax(QK^T)V in a transposed (feature-major) layout, and a partial output
projection against its row-slice of Wo. Partial outputs are summed on host.

Layout trick: all activations are kept feature-major (transposed), so every
matmul has its contraction dim on SBUF partitions and no on-device transpose
is ever needed. x/context are transposed on host; the output is produced
transposed and un-transposed on host.

Softmax: logits are small (|logit| < ~3), so exp() without max-subtraction is
numerically safe. The softmax denominator is obtained for free by augmenting
V with a ones-column (M=65 in the PV matmul): PSUM row 64 accumulates
sum(exp). Normalization happens on the tiny [64, 512] PV output, not on the
[2048, 2048] attention matrix.
"""

import sys

if "/opt/trn_rl_repo" not in sys.path:
    sys.path.insert(0, "/opt/trn_rl_repo")

import ml_dtypes
import numpy as np

import concourse.bass as bass
import concourse.mybir as mybir
import concourse.tile as tile
from concourse import bacc
from concourse.bass_utils import run_bass_kernel_spmd

HEADS = 16
DH = 64
QD = 1024  # query/context feature dim
NN = 2048  # query tokens
MM = 2048  # context tokens
NCORES = 8
HPC = HEADS // (NCORES // 2)  # 4 heads per core
HD = HPC * DH  # 256 inner cols per core

BF = mybir.dt.bfloat16
F32 = mybir.dt.float32

_CACHE = {}


def _build(debug_taps=False):
    nc = bacc.Bacc("TRN2", target_bir_lowering=False, debug=False)
    xT = nc.declare_dram_parameter("xT", [QD, NN], BF, isOutput=False)
    cT = nc.declare_dram_parameter("cT", [QD, MM], BF, isOutput=False)
    wq = nc.declare_dram_parameter("wq", [QD, HD], BF, isOutput=False)
    wk = nc.declare_dram_parameter("wk", [QD, HD], BF, isOutput=False)
    wv = nc.declare_dram_parameter("wv", [QD, HD], BF, isOutput=False)
    wo = nc.declare_dram_parameter("wo", [HD, QD], BF, isOutput=False)
    out = nc.declare_dram_parameter("out", [QD, NN], F32, isOutput=True)
    taps = None
    if debug_taps:
        taps = {
            "dq": nc.declare_dram_parameter("dq", [128, 2, NN], BF, isOutput=True),
            "dk": nc.declare_dram_parameter("dk", [128, 2, MM], BF, isOutput=True),
            "dv": nc.declare_dram_parameter("dv", [128, MM // 128, HPC, DH + 1], BF, isOutput=True),
            "dpvs": nc.declare_dram_parameter("dpvs", [128, 2, NN], BF, isOutput=True),
        }

    with tile.TileContext(nc) as tc:
        _emit(tc, xT, cT, wq, wk, wv, wo, out, taps)
    nc.compile()
    return nc


def _emit(tc, xT, cT, wq, wk, wv, wo, out, taps=None):
    nc = tc.nc
    Exp = mybir.ActivationFunctionType.Exp
    mult = mybir.AluOpType.mult
    KT = QD // 128  # 8 contraction tiles for projections
    TT = MM // 128  # 16 context-token tiles
    IB = NN // 512  # 4 query-column blocks

    from contextlib import ExitStack
    ctx = ExitStack()
    persist = ctx.enter_context(tc.tile_pool(name="persist", bufs=1))
    xs = persist.tile([128, KT, NN], BF, tag="xs")
    cs = persist.tile([128, KT, MM], BF, tag="cs")
    wqs = persist.tile([128, KT, HD], BF, tag="wqs")
    wks = persist.tile([128, KT, HD], BF, tag="wks")
    wvs = persist.tile([128, KT, HD], BF, tag="wvs")
    wos = persist.tile([128, 2, QD], BF, tag="wos")
    qs = persist.tile([128, 2, NN], BF, tag="qs")  # qT: [head-pair, tokens]
    ks = persist.tile([128, 2, MM], BF, tag="ks")
    vs = persist.tile([128, TT, HPC, DH + 1], BF, tag="vs")  # v + ones col
    pvs = persist.tile([128, 2, NN], BF, tag="pvs")  # normalized attnV^T

    big = ctx.enter_context(tc.tile_pool(name="big_ps", bufs=2, space="PSUM"))
    pvp = ctx.enter_context(tc.tile_pool(name="pv_ps", bufs=4, space="PSUM"))
    expp = ctx.enter_context(tc.tile_pool(name="expp", bufs=4))
    outp = ctx.enter_context(tc.tile_pool(name="outp", bufs=4))
    nrm = ctx.enter_context(tc.tile_pool(name="nrm", bufs=4))

    # ---- loads ----
    for k in range(KT):
        nc.sync.dma_start(xs[:, k, :], xT[k * 128:(k + 1) * 128, :])
        nc.sync.dma_start(cs[:, k, :], cT[k * 128:(k + 1) * 128, :])
        nc.sync.dma_start(wqs[:, k, :], wq[k * 128:(k + 1) * 128, :])
        nc.sync.dma_start(wks[:, k, :], wk[k * 128:(k + 1) * 128, :])
        nc.sync.dma_start(wvs[:, k, :], wv[k * 128:(k + 1) * 128, :])
    for t in range(2):
        nc.sync.dma_start(wos[:, t, :], wo[t * 128:(t + 1) * 128, :])
    nc.gpsimd.memset(vs[:, :, :, DH:DH + 1], 1.0)

    # ---- q/k projections (feature-major out: [j, tokens]) ----
    for dst, w, src in ((qs, wqs, xs), (ks, wks, cs)):
        for jb in range(2):
            for ib2 in range(2):
                ps = big.tile([128, 1024], F32, tag="big")
                for k in range(KT):
                    for i01 in range(2):
                        nc.tensor.matmul(
                            ps[:, i01 * 512:(i01 + 1) * 512],
                            lhsT=w[:, k, jb * 128:(jb + 1) * 128],
                            rhs=src[:, k, ib2 * 1024 + i01 * 512:ib2 * 1024 + (i01 + 1) * 512],
                            start=(k == 0),
                            stop=(k == KT - 1),
                        )
                nc.vector.tensor_copy(dst[:, jb, ib2 * 1024:(ib2 + 1) * 1024], ps[:, :])

    # ---- v projection (token-major out: [tokens, hd]) ----
    for tt in range(TT):
        ps = big.tile([128, HPC, DH], F32, tag="big")
        for k in range(KT):
            nc.tensor.matmul(
                ps[:, :, :],
                lhsT=cs[:, k, tt * 128:(tt + 1) * 128],
                rhs=wvs[:, k, :],
                start=(k == 0),
                stop=(k == KT - 1),
            )
        nc.vector.tensor_copy(vs[:, tt, :, 0:DH], ps[:, :, :])

    # ---- attention: per head-pair, per query-column pair ----
    for hp in range(2):
        for ib2 in range(2):
            pvt = {}
            for h01 in range(2):
                for i01 in range(2):
                    pvt[(h01, i01)] = pvp.tile([DH + 1, 512], F32, tag="pv", name="pv")
            for tt in range(TT):
                qk0 = big.tile([128, 1024], F32, tag="big")
                qk1 = big.tile([128, 1024], F32, tag="big")
                for i01 in range(2):
                    c0 = ib2 * 1024 + i01 * 512
                    nc.tensor.matmul(
                        qk0[:, i01 * 512:(i01 + 1) * 512],
                        lhsT=ks[0:64, hp, tt * 128:(tt + 1) * 128],
                        rhs=qs[0:64, hp, c0:c0 + 512],
                        start=True, stop=True,
                        tile_position=(0, 0),
                    )
                    nc.tensor.matmul(
                        qk1[:, i01 * 512:(i01 + 1) * 512],
                        lhsT=ks[64:128, hp, tt * 128:(tt + 1) * 128],
                        rhs=qs[64:128, hp, c0:c0 + 512],
                        start=True, stop=True,
                        tile_position=(64, 0),
                    )
                e0 = expp.tile([128, 1024], BF, tag="exp")
                nc.scalar.activation(e0[:, :], qk0[:, :], Exp, scale=0.125)
                e1 = expp.tile([128, 1024], BF, tag="exp")
                nc.scalar.activation(e1[:, :], qk1[:, :], Exp, scale=0.125)
                for i01 in range(2):
                    nc.tensor.matmul(
                        pvt[(0, i01)][:, :],
                        lhsT=vs[:, tt, 2 * hp, :],
                        rhs=e0[:, i01 * 512:(i01 + 1) * 512],
                        start=(tt == 0), stop=(tt == TT - 1),
                    )
                    nc.tensor.matmul(
                        pvt[(1, i01)][:, :],
                        lhsT=vs[:, tt, 2 * hp + 1, :],
                        rhs=e1[:, i01 * 512:(i01 + 1) * 512],
                        start=(tt == 0), stop=(tt == TT - 1),
                    )
            # normalize: pv[0:64] / pv[64] , write bf16 into pvs
            for h01 in range(2):
                for i01 in range(2):
                    p = pvt[(h01, i01)]
                    c0 = ib2 * 1024 + i01 * 512
                    rc = nrm.tile([1, 512], F32, tag="rc")
                    nc.vector.reciprocal(rc[:, :], p[64:65, :])
                    rep = nrm.tile([64, 512], F32, tag="rep")
                    nc.gpsimd.partition_broadcast(rep[:, :], rc[:, :])
                    nc.vector.tensor_tensor(
                        pvs[h01 * 64:(h01 + 1) * 64, hp, c0:c0 + 512],
                        p[0:64, :],
                        rep[:, :],
                        mult,
                    )

    # ---- output projection (partial; host sums across head-groups) ----
    for ib in range(IB):
        for ob in range(QD // 128):
            fp = big.tile([128, 512], F32, tag="big")
            for t2 in range(2):
                nc.tensor.matmul(
                    fp[:, :],
                    lhsT=wos[:, t2, ob * 128:(ob + 1) * 128],
                    rhs=pvs[:, t2, ib * 512:(ib + 1) * 512],
                    start=(t2 == 0), stop=(t2 == 1),
                )
            ot = outp.tile([128, 512], F32, tag="ot")
            nc.vector.tensor_copy(ot[:, :], fp[:, :])
            nc.sync.dma_start(out[ob * 128:(ob + 1) * 128, ib * 512:(ib + 1) * 512], ot[:, :])
    if taps is not None:
        nc.sync.dma_start(taps["dq"][:, :, :], qs[:, :, :])
        nc.sync.dma_start(taps["dk"][:, :, :], ks[:, :, :])
        nc.sync.dma_start(taps["dv"][:, :, :, :], vs[:, :, :, :])
        nc.sync.dma_start(taps["dpvs"][:, :, :], pvs[:, :, :])
    ctx.close()


def _inputs_for_core(c, x, context, Wq, Wk, Wv, Wo):
    bf = ml_dtypes.bfloat16
    b, g = c // (NCORES // 2), c % (NCORES // 2)
    sl = slice(g * HD, (g + 1) * HD)
    key = ("xc", b)
    if key not in _CACHE:
        _CACHE[key] = (
            np.ascontiguousarray(x[b].T).astype(bf),
            np.ascontiguousarray(context[b].T).astype(bf),
        )
    xTb, cTb = _CACHE[key]
    return {
        "xT": xTb,
        "cT": cTb,
        "wq": np.ascontiguousarray(Wq[:, sl]).astype(bf),
        "wk": np.ascontiguousarray(Wk[:, sl]).astype(bf),
        "wv": np.ascontiguousarray(Wv[:, sl]).astype(bf),
        "wo": np.ascontiguousarray(Wo[sl, :]).astype(bf),
    }


def kernel(x, context, Wq, Wk, Wv, Wo, bo):
    x = np.asarray(x, np.float32)
    context = np.asarray(context, np.float32)
    if "nc" not in _CACHE:
        _CACHE["nc"] = _build()
    _CACHE.pop(("xc", 0), None)
    _CACHE.pop(("xc", 1), None)
    nc = _CACHE["nc"]
    in_maps = [
        _inputs_for_core(c, x, context, np.asarray(Wq), np.asarray(Wk),
                         np.asarray(Wv), np.asarray(Wo))
        for c in range(NCORES)
    ]
    res = run_bass_kernel_spmd(nc, in_maps, list(range(NCORES))).results
    B = x.shape[0]
    G = NCORES // B
    outp = np.empty((B, NN, QD), np.float32)
    for b in range(B):
        acc = res[b * G]["out"].astype(np.float32)
        for g in range(1, G):
            acc = acc + res[b * G + g]["out"]
        outp[b] = acc.T + np.asarray(bo, np.float32)[None, :]
    return outp


# revision 25
# speedup vs baseline: 2.2603x; 2.2603x over previous
"""CrossAttention (B=2, N=M=2048, 16 heads x 64) on 8 TRN2 NeuronCores.

Sharding: data-parallel over batch (2) x tensor-parallel over heads (4 per
core). Each core computes q/k/v projections for its 4 heads, streaming
softmax(QK^T)V in a transposed (feature-major) layout, and a partial output
projection against its row-slice of Wo. Partial outputs are summed on host.

Layout trick: all activations are kept feature-major (transposed), so every
matmul has its contraction dim on SBUF partitions and no on-device transpose
is ever needed. x/context are transposed on host; the output is produced
transposed and un-transposed on host.

Softmax: logits are small (|logit| < ~3), so exp() without max-subtraction is
numerically safe. The softmax denominator is obtained for free by augmenting
V with a ones-column (M=65 in the PV matmul): PSUM row 64 accumulates
sum(exp). Normalization happens on the tiny [64, 512] PV output, not on the
[2048, 2048] attention matrix.
"""

import sys

if "/opt/trn_rl_repo" not in sys.path:
    sys.path.insert(0, "/opt/trn_rl_repo")

import ml_dtypes
import numpy as np

import concourse.bass as bass
import concourse.mybir as mybir
import concourse.tile as tile
from concourse import bacc
from concourse.bass_utils import run_bass_kernel_spmd

HEADS = 16
DH = 64
QD = 1024  # query/context feature dim
NN = 2048  # query tokens
MM = 2048  # context tokens
NCORES = 8
HPC = HEADS // (NCORES // 2)  # 4 heads per core
HD = HPC * DH  # 256 inner cols per core

BF = mybir.dt.bfloat16
F32 = mybir.dt.float32

_CACHE = {}


def _build(debug_taps=False, phases="all", repeat=1):
    nc = bacc.Bacc("TRN2", target_bir_lowering=False, debug=False)
    xT = nc.declare_dram_parameter("xT", [QD, NN], BF, isOutput=False)
    cT = nc.declare_dram_parameter("cT", [QD, MM], BF, isOutput=False)
    wq = nc.declare_dram_parameter("wq", [QD, HD], BF, isOutput=False)
    wk = nc.declare_dram_parameter("wk", [QD, HD], BF, isOutput=False)
    wv = nc.declare_dram_parameter("wv", [QD, HD], BF, isOutput=False)
    wo = nc.declare_dram_parameter("wo", [HD, QD], BF, isOutput=False)
    out = nc.declare_dram_parameter("out", [QD, NN], F32, isOutput=True)
    taps = None
    if debug_taps:
        taps = {
            "dq": nc.declare_dram_parameter("dq", [128, 2, NN], BF, isOutput=True),
            "dk": nc.declare_dram_parameter("dk", [128, 2, MM], BF, isOutput=True),
            "dv": nc.declare_dram_parameter("dv", [128, MM // 128, HPC, DH + 1], BF, isOutput=True),
            "dpvs": nc.declare_dram_parameter("dpvs", [128, 2, NN], BF, isOutput=True),
        }

    with tile.TileContext(nc) as tc:
        for _ in range(repeat):
            _emit(tc, xT, cT, wq, wk, wv, wo, out, taps, phases)
    nc.compile()
    return nc


def _emit(tc, xT, cT, wq, wk, wv, wo, out, taps=None, phases="all"):
    nc = tc.nc
    Exp = mybir.ActivationFunctionType.Exp
    mult = mybir.AluOpType.mult
    KT = QD // 128  # 8 contraction tiles for projections
    TT = MM // 128  # 16 context-token tiles
    IB = NN // 512  # 4 query-column blocks

    from contextlib import ExitStack
    ctx = ExitStack()
    persist = ctx.enter_context(tc.tile_pool(name="persist", bufs=1))
    xs = persist.tile([128, KT, NN], BF, tag="xs")
    cs = persist.tile([128, KT, MM], BF, tag="cs")
    wqs = persist.tile([128, KT, HD], BF, tag="wqs")
    wks = persist.tile([128, KT, HD], BF, tag="wks")
    wvs = persist.tile([128, KT, HD], BF, tag="wvs")
    wos = persist.tile([128, 2, QD], BF, tag="wos")
    qs = persist.tile([128, 2, NN], BF, tag="qs")  # qT: [head-pair, tokens]
    ks = persist.tile([128, 2, MM], BF, tag="ks")
    vs = persist.tile([128, TT, HPC, DH + 1], BF, tag="vs")  # v + ones col
    pvs = persist.tile([128, 2, NN], BF, tag="pvs")  # normalized attnV^T

    qkp = ctx.enter_context(tc.tile_pool(name="qk_ps", bufs=2, space="PSUM"))
    pvp = ctx.enter_context(tc.tile_pool(name="pv_ps", bufs=2, space="PSUM"))
    projp = ctx.enter_context(tc.tile_pool(name="proj_ps", bufs=2, space="PSUM"))
    expp = ctx.enter_context(tc.tile_pool(name="expp", bufs=33))
    outp = ctx.enter_context(tc.tile_pool(name="outp", bufs=2))
    nrm = ctx.enter_context(tc.tile_pool(name="nrm", bufs=4))

    # ---- loads: weights first, x/context k-tiles interleaved ----
    for k in range(KT):
        nc.sync.dma_start(wqs[:, k, :], wq[k * 128:(k + 1) * 128, :])
        nc.sync.dma_start(wks[:, k, :], wk[k * 128:(k + 1) * 128, :])
        nc.sync.dma_start(wvs[:, k, :], wv[k * 128:(k + 1) * 128, :])
    for t in range(2):
        nc.sync.dma_start(wos[:, t, :], wo[t * 128:(t + 1) * 128, :])
    for k in range(KT):
        nc.sync.dma_start(xs[:, k, :], xT[k * 128:(k + 1) * 128, :])
        nc.sync.dma_start(cs[:, k, :], cT[k * 128:(k + 1) * 128, :])
    nc.gpsimd.memset(vs[:, :, :, DH:DH + 1], 1.0)

    do = lambda p: phases == "all" or p in phases
    if not do("proj"):
        for t in (qs, ks, pvs):
            nc.gpsimd.memset(t[:, :, :], 0.25)
        nc.gpsimd.memset(vs[:, :, :, 0:DH], 0.25)

    def qk_chain(jb, i4, dst, w, src):
        ps = projp.tile([128, 512], F32, tag="proj", name="ps")
        for k in range(KT):
            nc.tensor.matmul(
                ps[:, :],
                lhsT=w[:, k, jb * 128:(jb + 1) * 128],
                rhs=src[:, k, i4 * 512:(i4 + 1) * 512],
                start=(k == 0),
                stop=(k == KT - 1),
            )
        nc.vector.tensor_copy(dst[:, jb, i4 * 512:(i4 + 1) * 512], ps[:, :])

    def qk_proj(jb):
        # q/k projections for head-pair jb (feature-major out: [j, tokens])
        for dst, w, src in ((qs, wqs, xs), (ks, wks, cs)):
            for i4 in range(4):
                qk_chain(jb, i4, dst, w, src)

    def v_chain(tt):
        # v projection for one token tile (token-major out: [tokens, hd])
        ps = projp.tile([128, HPC, DH], F32, tag="proj", name="ps")
        for k in range(KT):
            nc.tensor.matmul(
                ps[:, :, :],
                lhsT=cs[:, k, tt * 128:(tt + 1) * 128],
                rhs=wvs[:, k, :],
                start=(k == 0),
                stop=(k == KT - 1),
            )
        nc.vector.tensor_copy(vs[:, tt, :, 0:DH], ps[:, :, :])

    def v_proj():
        for tt in range(TT):
            v_chain(tt)

    def final_proj(ib):
        for ob in range(QD // 128):
            fp = projp.tile([128, 512], F32, tag="proj", name="fp")
            for t2 in range(2):
                nc.tensor.matmul(
                    fp[:, :],
                    lhsT=wos[:, t2, ob * 128:(ob + 1) * 128],
                    rhs=pvs[:, t2, ib * 512:(ib + 1) * 512],
                    start=(t2 == 0), stop=(t2 == 1),
                )
            ot = outp.tile([128, 512], F32, tag="ot", name="ot")
            nc.vector.tensor_copy(ot[:, :], fp[:, :])
            nc.sync.dma_start(out[ob * 128:(ob + 1) * 128, ib * 512:(ib + 1) * 512], ot[:, :])

    def attn(hp, ib2, first=False, fillers=()):
        # QK^T + exp for all 16 token tiles (2-head row-packed, K=64).
        # `fillers` are deferred projection chains emitted one-per-token-tile
        # so the PE works on them while ACT streams the exp pass.
        fillers = list(fillers)
        es = {}
        for tt in range(TT):
            qk0 = qkp.tile([128, 1024], F32, tag="qk", name="qk0")
            qk1 = qkp.tile([128, 1024], F32, tag="qk", name="qk1")
            for i01 in range(2):
                c0 = ib2 * 1024 + i01 * 512
                nc.tensor.matmul(
                    qk0[:, i01 * 512:(i01 + 1) * 512],
                    lhsT=ks[0:64, hp, tt * 128:(tt + 1) * 128],
                    rhs=qs[0:64, hp, c0:c0 + 512],
                    start=True, stop=True,
                    tile_position=(0, 0),
                )
                nc.tensor.matmul(
                    qk1[:, i01 * 512:(i01 + 1) * 512],
                    lhsT=ks[64:128, hp, tt * 128:(tt + 1) * 128],
                    rhs=qs[64:128, hp, c0:c0 + 512],
                    start=True, stop=True,
                    tile_position=(64, 0),
                )
            e0 = expp.tile([128, 1024], BF, tag="exp", name="e0")
            nc.scalar.activation(e0[:, :], qk0[:, :], Exp, scale=0.125)
            e1 = expp.tile([128, 1024], BF, tag="exp", name="e1")
            nc.scalar.activation(e1[:, :], qk1[:, :], Exp, scale=0.125)
            es[(tt, 0)], es[(tt, 1)] = e0, e1
            if fillers:
                fillers.pop(0)()
        while fillers:
            fillers.pop(0)()
        # PV + rowsum (M=65 augmented V), then normalize
        for h01 in range(2):
            pvt = [pvp.tile([DH + 1, 512], F32, tag="pv", name="pv") for _ in range(2)]
            for tt in range(TT):
                for i01 in range(2):
                    nc.tensor.matmul(
                        pvt[i01][:, :],
                        lhsT=vs[:, tt, 2 * hp + h01, :],
                        rhs=es[(tt, h01)][:, i01 * 512:(i01 + 1) * 512],
                        start=(tt == 0), stop=(tt == TT - 1),
                    )
            for i01 in range(2):
                p = pvt[i01]
                c0 = ib2 * 1024 + i01 * 512
                rc = nrm.tile([1, 512], F32, tag="rc", name="rc")
                nc.vector.reciprocal(rc[:, :], p[64:65, :])
                rep = nrm.tile([64, 512], F32, tag="rep", name="rep")
                nc.gpsimd.partition_broadcast(rep[:, :], rc[:, :])
                nc.vector.tensor_tensor(
                    pvs[h01 * 64:(h01 + 1) * 64, hp, c0:c0 + 512],
                    p[0:64, :],
                    rep[:, :],
                    mult,
                )

    if do("proj"):
        qk_proj(0)
    if do("attn"):
        if do("proj"):
            vfill = [(lambda t=t: v_chain(t)) for t in range(TT)]
            qkfill = [
                (lambda i=i, d=d, w=w, s=s: qk_chain(1, i, d, w, s))
                for d, w, s in ((qs, wqs, xs), (ks, wks, cs))
                for i in range(4)
            ]
        else:
            vfill, qkfill = [], []
        attn(0, 0, fillers=vfill)
        attn(0, 1, fillers=qkfill)
        for ib2 in range(2):
            attn(1, ib2)
            if do("final"):
                final_proj(2 * ib2)
                final_proj(2 * ib2 + 1)
    elif do("final"):
        for ib in range(IB):
            final_proj(ib)
    if do("proj") and not do("attn"):
        qk_proj(1)
        v_proj()
    if taps is not None:
        nc.sync.dma_start(taps["dq"][:, :, :], qs[:, :, :])
        nc.sync.dma_start(taps["dk"][:, :, :], ks[:, :, :])
        nc.sync.dma_start(taps["dv"][:, :, :, :], vs[:, :, :, :])
        nc.sync.dma_start(taps["dpvs"][:, :, :], pvs[:, :, :])
    ctx.close()


def _inputs_for_core(c, x, context, Wq, Wk, Wv, Wo):
    bf = ml_dtypes.bfloat16
    b, g = c // (NCORES // 2), c % (NCORES // 2)
    sl = slice(g * HD, (g + 1) * HD)
    key = ("xc", b)
    if key not in _CACHE:
        _CACHE[key] = (
            np.ascontiguousarray(x[b].T).astype(bf),
            np.ascontiguousarray(context[b].T).astype(bf),
        )
    xTb, cTb = _CACHE[key]
    return {
        "xT": xTb,
        "cT": cTb,
        "wq": np.ascontiguousarray(Wq[:, sl]).astype(bf),
        "wk": np.ascontiguousarray(Wk[:, sl]).astype(bf),
        "wv": np.ascontiguousarray(Wv[:, sl]).astype(bf),
        "wo": np.ascontiguousarray(Wo[sl, :]).astype(bf),
    }


def kernel(x, context, Wq, Wk, Wv, Wo, bo):
    x = np.asarray(x, np.float32)
    context = np.asarray(context, np.float32)
    if "nc" not in _CACHE:
        _CACHE["nc"] = _build()
    _CACHE.pop(("xc", 0), None)
    _CACHE.pop(("xc", 1), None)
    nc = _CACHE["nc"]
    in_maps = [
        _inputs_for_core(c, x, context, np.asarray(Wq), np.asarray(Wk),
                         np.asarray(Wv), np.asarray(Wo))
        for c in range(NCORES)
    ]
    res = run_bass_kernel_spmd(nc, in_maps, list(range(NCORES))).results
    B = x.shape[0]
    G = NCORES // B
    outp = np.empty((B, NN, QD), np.float32)
    for b in range(B):
        acc = res[b * G]["out"].astype(np.float32)
        for g in range(1, G):
            acc = acc + res[b * G + g]["out"]
        outp[b] = acc.T + np.asarray(bo, np.float32)[None, :]
    return outp
